# revision 33
# baseline (speedup 1.0000x reference)
"""BinaryConnectNet forward pass on 8 Trainium2 NeuronCores (data parallel).

Batch 512 -> 64 per core; binarized weight signs baked host-side and
replicated; shift-BN global batch statistics all-reduced across the 8 cores.

Per-core design (v2, engine-balanced):
  conv1: host im2col packs IMAGE PAIRS (p, p+32) into K=54 rows; one PE
    matmul [54K,128M,512N] computes half an image for two images at once
    (all 128 PSUM partitions used). Maxpool x-step on DVE (reduce),
    y-step on Pool (tensor_tensor max), clip+S1 on DVE (accum_out),
    S2 via ACT Square (accum_out). c1/c2 pad rings zeroed border-only.
  depthwise 3x3 + residual: PE block-diagonal matmuls, 9 taps in PSUM.
  1x1 convs: PE matmuls; clip carries S1 via accum_out; S2 via DVE
    tensor_tensor_reduce (x*x) -> per-chunk column.
  BN stats: per-block single AllReduce (block3/4 pack both groups into
    one [128,4] AR instead of two).  AP2 shift via Ln/round/Exp on ACT.
  block4: clip writes c4 bf16 directly (no ACT copy); t4 PSUM->SBUF
    copies on Pool; BN3 apply fused into the c3 stream (ACT relu).
  FC head: per (kg,pq) pixel-slice relu (alternating ACT / DVE 2-op)
    interleaved with the 64 accumulating matmuls for those pixels.
"""
import os
import numpy as np
import ml_dtypes

import concourse.bass as bass
import concourse.bacc as bacc
import concourse.tile as tile
import concourse.mybir as mybir
from concourse import bass_utils

N_CORES = 8
B_CORE = 64
LN2 = float(np.log(2.0))
EPS = 1e-5
F32 = mybir.dt.float32
F32R = mybir.dt.float32r
BF16 = mybir.dt.bfloat16
I32 = mybir.dt.int32
U32 = mybir.dt.uint32
AO = mybir.AluOpType
AF = mybir.ActivationFunctionType
AX = mybir.AxisListType

_CACHE = {}


# ----------------------------------------------------------------- host prep

def _host_prep(x, w1, w21, w31, w41, w22, w32, w42, wfc):
    sgn = lambda w: np.where(np.asarray(w) >= 0, 1.0, -1.0).astype(np.float32)

    xp = np.pad(np.asarray(x, np.float32), ((0, 0), (0, 0), (1, 1), (1, 1)))
    cols = []
    for ci in range(3):
        for ky in range(3):
            for kx in range(3):
                cols.append(xp[:, ci, ky:ky + 32, kx:kx + 32])
    cols = np.stack(cols, 0).reshape(27, N_CORES, 2, 32, 1024)
    # pair images (p, p+32): rows 0-26 = img p patches, 27-53 = img p+32
    xcol2 = np.concatenate([cols[:, :, 0], cols[:, :, 1]], axis=0)
    xcol2 = np.ascontiguousarray(
        xcol2.transpose(1, 0, 2, 3)).reshape(N_CORES, 54, 32 * 1024)

    w1t = sgn(w1).reshape(64, 27).T                       # [27, 64]
    w2t = np.zeros((54, 128), np.float32)
    w2t[:27, :64] = w1t
    w2t[27:, 64:] = w1t

    def diag_pack(wdw, nch):
        s = sgn(wdw).reshape(nch, 9).copy()
        s[:, 4] += 1.0  # fold residual: t = h + dw(h)
        groups = []
        if nch == 64:
            d = np.zeros((128, 9, 128), np.float32)
            for p in range(128):
                d[p, :, p] = s[p % 64]
            groups.append(d.reshape(128, 9 * 128))
        else:
            for g in range(nch // 128):
                d = np.zeros((128, 9, 128), np.float32)
                for p in range(128):
                    d[p, :, p] = s[g * 128 + p]
                groups.append(d.reshape(128, 9 * 128))
        return np.stack(groups)

    d2 = diag_pack(w21, 64)[0]
    d3 = diag_pack(w31, 64)[0]
    d4 = diag_pack(w41, 256)                              # [2, 128, 1152]

    w22t = np.ascontiguousarray(sgn(w22)[:, :, 0, 0].T)   # [64, 64]
    w22t = np.concatenate([w22t, w22t], 0)                # [128, 64]
    w32t = np.ascontiguousarray(sgn(w32)[:, :, 0, 0].T)   # [64, 256]
    w32t = np.concatenate([w32t, w32t], 0)                # [128, 256]
    w42t = np.ascontiguousarray(
        sgn(w42)[:, :, 0, 0].T).reshape(2, 128, 256)      # [kg][ci, 256co]

    wf = sgn(wfc).reshape(10, 256, 256)                   # [o, c, pix]
    wfct = np.ascontiguousarray(
        wf.transpose(1, 2, 0)).reshape(2, 128, 2560).astype(
            ml_dtypes.bfloat16)                           # [kg][ci, pix*10+o]
    return xcol2, w2t, d2, d3, d4, w22t, w32t, w42t, wfct


# ------------------------------------------------------------ device pieces

def _emit_dw(nc, ps, hpad_view, diag_sb, nb_img, psum_tag):
    """Depthwise(+identity) over padded images [128, nb_img, 18, 18].
    Returns psum tile [128, nb_img, 16, 16] (full rectangles only)."""
    p = ps.tile([128, nb_img, 16, 16], F32, tag=psum_tag)
    order = [4, 0, 1, 2, 3, 5, 6, 7, 8]
    for i, t in enumerate(order):
        dy, dx = t // 3, t % 3
        nc.tensor.matmul(
            p[:], diag_sb[:, t, :],
            hpad_view[:, :, dy:dy + 16, dx:dx + 16],
            start=(i == 0), stop=(i == 8))
    return p


def _emit_ab(nc, sm, s1_ap, s2_ap, n_tot, gamma, beta, tag):
    """(sum x, sum x^2) [P,1] APs -> (a, b) [P,1] BN coefficients."""
    P = s1_ap.shape[0]
    mu = sm.tile([P, 1], F32, tag=tag + "mu")
    nc.vector.tensor_scalar(mu[:], s1_ap, 1.0 / n_tot, None, op0=AO.mult)
    exx = sm.tile([P, 1], F32, tag=tag + "ex")
    nc.vector.tensor_scalar(exx[:], s2_ap, 1.0 / n_tot, None, op0=AO.mult)
    musq = sm.tile([P, 1], F32, tag=tag + "m2")
    nc.vector.tensor_tensor(musq[:], mu[:], mu[:], op=AO.mult)
    var = sm.tile([P, 1], F32, tag=tag + "va")
    nc.vector.tensor_tensor(var[:], exx[:], musq[:], op=AO.subtract)
    eps_t = sm.tile([P, 1], F32, tag=tag + "ep")
    nc.vector.memset(eps_t[:], EPS)
    lg = sm.tile([P, 1], F32, tag=tag + "lg")
    nc.scalar.activation(lg[:], var[:], AF.Ln, bias=eps_t[:], scale=1.0)
    t = sm.tile([P, 1], F32, tag=tag + "t")
    nc.vector.tensor_scalar(t[:], lg[:], -0.5 / LN2, None, op0=AO.mult)
    ti = sm.tile([P, 1], I32, tag=tag + "ti")
    nc.vector.tensor_copy(ti[:], t[:])        # fp32->int32 rounds to nearest
    tf = sm.tile([P, 1], F32, tag=tag + "tf")
    nc.vector.tensor_copy(tf[:], ti[:])
    zb = sm.tile([P, 1], F32, tag=tag + "zb")
    nc.vector.memset(zb[:], 0.0)
    sh = sm.tile([P, 1], F32, tag=tag + "sh")
    nc.scalar.activation(sh[:], tf[:], AF.Exp, bias=zb[:], scale=LN2)
    a = sm.tile([P, 1], F32, tag=tag + "a")
    nc.vector.tensor_tensor(a[:], sh[:], gamma, op=AO.mult)
    b = sm.tile([P, 1], F32, tag=tag + "b")
    nc.vector.scalar_tensor_tensor(b[:], a[:], mu[:], beta,
                                   op0=AO.mult, op1=AO.subtract)
    nc.vector.tensor_scalar(b[:], b[:], -1.0, None, op0=AO.mult)
    return a, b


def _allreduce(nc, dram, src_ap, shape, tag):
    ar_in = dram.tile(list(shape), F32, tag=tag + "i")
    ar_out = dram.tile(list(shape), F32, tag=tag + "o")
    nc.gpsimd.dma_start(out=ar_in[:], in_=src_ap)
    nc.gpsimd.collective_compute(
        "AllReduce", AO.add, replica_groups=[list(range(N_CORES))],
        ins=[ar_in.opt()], outs=[ar_out.opt()])
    return ar_out


def _sum_chunks(nc, sm, sc1, sc2, nch, tag, out=None, col=0):
    """sc1/sc2 [P, nch] chunk sums -> packed (S1, S2) cols of `out`."""
    P = sc1.shape[0]
    pk = out if out is not None else sm.tile([P, 2], F32, tag=tag + "pk")
    nc.vector.tensor_reduce(pk[:, col:col + 1], sc1[:, 0:nch],
                            axis=AX.X, op=AO.add)
    nc.vector.tensor_reduce(pk[:, col + 1:col + 2], sc2[:, 0:nch],
                            axis=AX.X, op=AO.add)
    return pk


# ------------------------------------------------------------- device build

def build(debug=False):
    nc = bacc.Bacc("TRN2", target_bir_lowering=False, debug=False,
                   num_devices=N_CORES)
    din = {}

    def dd(name, shape, dtype=F32R):
        din[name] = nc.dram_tensor(name, list(shape), dtype,
                                   kind="ExternalInput")

    dd("xcol2", [54, 32 * 1024])
    dd("w2t", [54, 128])
    dd("d2", [128, 9 * 128])
    dd("d3", [128, 9 * 128])
    dd("d4", [2, 128, 9 * 128])
    dd("w22t", [128, 64])
    dd("w32t", [128, 256])
    dd("w42t", [2, 128, 256])
    dd("wfct", [2, 128, 2560], BF16)
    dd("gb", [128, 10], F32)
    dd("gb2", [128, 4], F32)
    out_d = nc.dram_tensor("out", [10, B_CORE], F32, kind="ExternalOutput")

    dbg = {}
    if debug:
        for name, shape, dt in [
                ("c1", [128, 32, 18, 18], F32), ("sg1", [64, 2], F32),
                ("ab1", [128, 2], F32), ("h1", [128, 32, 18, 18], F32),
                ("c2", [128, 32, 18, 18], F32), ("h2", [128, 32, 18, 18], F32),
                ("c3", [2, 128, 16384], F32),
                ("c4", [2, 128, 16384], F32), ("h4", [2, 128, 16384], F32)]:
            dbg[name] = nc.dram_tensor("dbg_" + name, shape, dt,
                                       kind="ExternalOutput")

    with tile.TileContext(nc) as tc:
        with tc.tile_pool(name="wts", bufs=1) as wts, \
             tc.tile_pool(name="sb", bufs=1) as sb, \
             tc.tile_pool(name="sm", bufs=2) as sm, \
             tc.tile_pool(name="scr", bufs=1) as scr, \
             tc.tile_pool(name="xin", bufs=2) as xin, \
             tc.tile_pool(name="cho", bufs=4) as cho, \
             tc.tile_pool(name="ps", bufs=3, space="PSUM") as ps, \
             tc.tile_pool(name="psf", bufs=1, space="PSUM") as psf, \
             tc.tile_pool(name="dram", bufs=1, space="DRAM") as dram:
            _body(nc, tc, wts, sb, sm, scr, xin, cho, ps, psf, dram,
                  din, out_d, dbg)
    nc.compile()
    return nc


def _zero_border(nc, t, nimg):
    """Zero only the 1-px pad ring of t [128, nimg, 18, 18] (on Pool)."""
    v = t[:]
    nc.gpsimd.memset(v[:, :, 0:18:17, :].bitcast(U32), 0)
    nc.gpsimd.memset(v[:, :, 1:17, 0:18:17].bitcast(U32), 0)


def _body(nc, tc, wts, sb, sm, scr, xin, cho, ps, psf, dram,
          din, out_d, dbg):
    # ---------- weights (ddw slot chains d2 -> d3 -> d4)
    def wload(name, shape, dtype=F32R, dma=True, tag=None):
        t = wts.tile(list(shape), dtype, tag=tag or name)
        if dma:
            nc.sync.dma_start(out=t, in_=din[name].ap())
        return t

    w2t = wload("w2t", [54, 128])
    d2w = wload("d2", [128, 2, 9, 128], tag="ddw", dma=False)
    nc.sync.dma_start(out=d2w[:, 0], in_=din["d2"].ap())
    d2 = d2w[:, 0]
    w22t = wload("w22t", [128, 64])
    w32t = wload("w32t", [128, 256])
    w42t = wload("w42t", [128, 2, 256], dma=False)
    gb = wload("gb", [128, 10], F32)
    gb2 = wload("gb2", [128, 4], F32)
    for g in range(2):
        nc.sync.dma_start(out=w42t[:, g], in_=din["w42t"].ap()[g])

    # ---------- stage A: conv1 + maxpool2 + clip -> c1 padded f32r
    c1 = sb.tile([128, 32, 18, 18], F32R, tag="chainA")
    _zero_border(nc, c1, 32)
    sc1a = sm.tile([128, 64], F32, tag="sc1a")
    sc2a = sm.tile([128, 64], F32, tag="sc2a")
    for p in range(32):
        if p % 2 == 0:
            xc = xin.tile([54, 2, 2, 512], F32R, tag="xc")
            nc.sync.dma_start(
                out=xc,
                in_=din["xcol2"].ap()[:, p * 1024:(p + 2) * 1024])
        for half in range(2):
            pc = ps.tile([128, 512], F32, tag="pdw")
            nc.tensor.matmul(pc[:], w2t[:], xc[:, p % 2, half],
                             start=True, stop=True)
            pcv = pc[:].rearrange("p (y xp two) -> p y xp two", y=16, two=2)
            px = sm.tile([128, 16, 16], F32, tag="px")
            nc.vector.tensor_reduce(px[:], pcv, axis=AX.X, op=AO.max)
            pxv = px[:].rearrange("p (yp two) x -> p yp two x", two=2)
            ymx = sm.tile([128, 8, 16], F32, tag="ymx")
            nc.vector.tensor_tensor(ymx[:], pxv[:, :, 0, :],
                                    pxv[:, :, 1, :], op=AO.max)
            col = p * 2 + half
            ymc = sm.tile([128, 8, 16], F32R, tag="ymc")
            nc.vector.tensor_scalar(ymc[:], ymx[:], -128.0, 127.0,
                                    op0=AO.max, op1=AO.min)
            dst = c1[:, p, 1 + half * 8:9 + half * 8, 1:17]
            nc.scalar.activation(dst, ymc[:], AF.Copy,
                                 accum_out=sc1a[:, col:col + 1])
            sqs = sm.tile([128, 8, 16], F32, tag="sqs")
            nc.scalar.activation(sqs[:], ymc[:], AF.Square,
                                 accum_out=sc2a[:, col:col + 1])
    if dbg:
        nc.sync.dma_start(out=dbg["c1"].ap(), in_=c1[:].bitcast(F32))

    # ---------- BN1 (stats fused above) -> apply in place
    h1 = _bn_apply_small(nc, sm, dram, c1, sc1a, sc2a, 64,
                         gb[:, 0:1], gb[:, 1:2], "bn1", dbg, "sg1", "ab1")
    if dbg:
        nc.sync.dma_start(out=dbg["h1"].ap(), in_=c1[:].bitcast(F32))

    # ---------- block2: dw2 + 1x1(64->64) -> c2 padded ; BN2 in place
    c2 = sb.tile([128, 32, 18, 18], F32R, tag="chainB")
    _zero_border(nc, c2, 32)
    sc1b = sm.tile([128, 16], F32, tag="sc1b")
    sc2b = sm.tile([128, 16], F32, tag="sc2b")
    for b0 in range(0, 32, 2):
        p = _emit_dw(nc, ps, h1[:, b0:b0 + 2], d2, 2, "pdw")
        t2 = cho.tile([128, 2, 16, 16], F32R, tag="t2")
        nc.scalar.copy(t2[:], p[:])
        ci = b0 // 2
        for bh in range(2):
            pu = ps.tile([64, 512], F32, tag="pu")
            nc.tensor.matmul(
                pu[:], w22t[bh * 64:(bh + 1) * 64, :],
                t2[bh * 64:(bh + 1) * 64].rearrange("p a b c -> p (a b c)"),
                start=True, stop=True)
            dst = c2[bh * 64:(bh + 1) * 64, b0:b0 + 2, 1:17, 1:17]
            nc.vector.tensor_scalar(
                dst, pu[:].rearrange("p (a b c) -> p a b c", a=2, b=16),
                -128.0, 127.0, op0=AO.max, op1=AO.min)
        iv = c2[:, b0:b0 + 2, 1:17, 1:17]
        nc.vector.tensor_reduce(sc1b[:, ci:ci + 1], iv, axis=AX.XYZ,
                                op=AO.add)
        s2s = scr.tile([128, 2, 16, 16], F32, tag="s2scr")
        nc.scalar.activation(s2s[:], iv, AF.Square,
                             accum_out=sc2b[:, ci:ci + 1])
    if dbg:
        nc.sync.dma_start(out=dbg["c2"].ap(), in_=c2[:].bitcast(F32))
    h2 = _bn_apply_small(nc, sm, dram, c2, sc1b, sc2b, 16,
                         gb[:, 2:3], gb[:, 3:4], "bn2", dbg, None, None)
    if dbg:
        nc.sync.dma_start(out=dbg["h2"].ap(), in_=c2[:].bitcast(F32))

    # load d3 into the freed ddw slot
    d3w = wts.tile([128, 2, 9, 128], F32R, tag="ddw")
    nc.sync.dma_start(out=d3w[:, 0], in_=din["d3"].ap())
    d3 = d3w[:, 0]

    # ---------- block3: dw3 + 1x1(64->256) -> c3 DRAM (both groups)
    c3_dram = dram.tile([2, 128, 16384], F32R, tag="c3d")
    sc13 = sm.tile([128, 2, 32], F32, tag="sc13")
    sc23 = sm.tile([128, 2, 32], F32, tag="sc23")
    for b0 in range(0, 32, 2):
        p = _emit_dw(nc, ps, h2[:, b0:b0 + 2], d3, 2, "pdw")
        t3 = cho.tile([128, 2, 16, 16], F32R, tag="t2")
        nc.scalar.copy(t3[:], p[:])
        for bh in range(2):
            b_abs = bh * 32 + b0
            ci = b_abs // 2
            for g in range(2):
                pu = ps.tile([128, 512], F32, tag="pu")
                nc.tensor.matmul(
                    pu[:], w32t[bh * 64:(bh + 1) * 64,
                                g * 128:(g + 1) * 128],
                    t3[bh * 64:(bh + 1) * 64]
                    .rearrange("p a b c -> p (a b c)"),
                    start=True, stop=True)
                cc = cho.tile([128, 512], F32R, tag="ccs")
                nc.vector.tensor_scalar(cc[:], pu[:], -128.0, 127.0,
                                        op0=AO.max, op1=AO.min)
                nc.vector.tensor_reduce(sc13[:, g, ci:ci + 1], cc[:],
                                        axis=AX.X, op=AO.add)
                s2s = scr.tile([128, 512], F32, tag="s2sc3")
                nc.scalar.activation(s2s[:], cc[:], AF.Square,
                                     accum_out=sc23[:, g, ci:ci + 1])
                nc.sync.dma_start(
                    out=c3_dram[g, :, b_abs * 256:(b_abs + 2) * 256],
                    in_=cc[:])
    if dbg:
        for g in range(2):
            nc.sync.dma_start(out=dbg["c3"].ap()[g],
                              in_=c3_dram[g].bitcast(F32))

    # single AllReduce for both groups: cols (S1g0, S2g0, S1g1, S2g1)
    pk3 = sm.tile([128, 4], F32, tag="pk3")
    for g in range(2):
        _sum_chunks(nc, sm, sc13[:, g], sc23[:, g], 32, f"s3{g}",
                    out=pk3, col=2 * g)
    ar3 = _allreduce(nc, dram, pk3[:], [128, 4], "ar3")
    sg3 = sm.tile([128, 4], F32, tag="sg3")
    nc.gpsimd.dma_start(out=sg3[:], in_=ar3[:])
    ab3 = []
    for g in range(2):
        ab3.append(_emit_ab(nc, sm, sg3[:, 2 * g:2 * g + 1],
                            sg3[:, 2 * g + 1:2 * g + 2], 131072,
                            gb[:, 4 + g:5 + g], gb[:, 6 + g:7 + g],
                            f"bn3g{g}"))

    # ---------- block4: stream c3 chunks, BN3 on the fly, dw4, 1x1 -> c4
    c4_g0 = sb.tile([128, 64, 16, 16], BF16, tag="chainB")
    c4_g1 = sb.tile([128, 64, 16, 16], BF16, tag="chainA")
    c4 = [c4_g0, c4_g1]
    d4 = wts.tile([128, 2, 9, 128], F32R, tag="ddw")
    for g in range(2):
        nc.sync.dma_start(out=d4[:, g], in_=din["d4"].ap()[g])
    h3c = []
    for g in range(2):
        for s in range(2):
            t = sb.tile([128, 2, 18, 18], F32R, tag=f"h3c{g}{s}")
            _zero_border(nc, t, 2)
            h3c.append(t)
    sc14 = sm.tile([128, 2, 32], F32, tag="sc14")
    sc24 = sm.tile([128, 2, 32], F32, tag="sc24")
    for b0 in range(0, 64, 2):
        ci = b0 // 2
        t4 = []
        for g in range(2):
            c3c = cho.tile([128, 512], F32R, tag="c3c")
            nc.sync.dma_start(out=c3c,
                              in_=c3_dram[g, :, b0 * 256:(b0 + 2) * 256])
            hp = h3c[g * 2 + (ci % 2)]
            nc.scalar.activation(
                hp[:, :, 1:17, 1:17],
                c3c[:].rearrange("p (a b c) -> p a b c", a=2, b=16),
                AF.Relu, bias=ab3[g][1][:], scale=ab3[g][0][:])
            p = _emit_dw(nc, ps, hp[:], d4[:, g], 2, "pdw")
            tg = cho.tile([128, 2, 16, 16], F32R, tag=f"t4_{g}")
            nc.vector.tensor_copy(tg[:], p[:])
            t4.append(tg)
        for mg in range(2):
            pu = ps.tile([128, 512], F32, tag="pu")
            for kg in range(2):
                nc.tensor.matmul(
                    pu[:], w42t[:, kg, mg * 128:(mg + 1) * 128],
                    t4[kg][:].rearrange("p a b c -> p (a b c)"),
                    start=(kg == 0), stop=(kg == 1))
            dst = c4[mg][:, b0:b0 + 2].rearrange("p a b c -> p (a b c)")
            nc.vector.tensor_scalar(
                dst, pu[:], -128.0, 127.0, op0=AO.max, op1=AO.min)
            nc.vector.tensor_reduce(sc14[:, mg, ci:ci + 1], dst,
                                    axis=AX.X, op=AO.add)
            s2s = scr.tile([128, 512], F32, tag="s2sc4")
            nc.scalar.activation(s2s[:], dst, AF.Square,
                                 accum_out=sc24[:, mg, ci:ci + 1])
    if dbg:
        for g in range(2):
            nc.gpsimd.dma_start(
                out=dbg["c4"].ap()[g],
                in_=c4[g][:].rearrange("p a b c -> p (a b c)"))

    # single AllReduce for both groups of BN4
    pk4 = sm.tile([128, 4], F32, tag="pk4")
    for g in range(2):
        _sum_chunks(nc, sm, sc14[:, g], sc24[:, g], 32, f"s4{g}",
                    out=pk4, col=2 * g)
    ar4 = _allreduce(nc, dram, pk4[:], [128, 4], "ar4")
    sg4 = sm.tile([128, 4], F32, tag="sg4")
    nc.gpsimd.dma_start(out=sg4[:], in_=ar4[:])
    ab4 = []
    for g in range(2):
        ab4.append(_emit_ab(nc, sm, sg4[:, 2 * g:2 * g + 1],
                            sg4[:, 2 * g + 1:2 * g + 2], 131072,
                            gb2[:, g:g + 1], gb2[:, 2 + g:3 + g],
                            f"bn4g{g}"))

    # ---------- FC head: per (kg,pq) relu pixel-slice, then its matmuls
    pf = psf.tile([10, 64], F32, tag="pf")
    n_mm = 0
    for kg in range(2):
        h4v = c4[kg][:].rearrange("p b y x -> p b (y x)")
        for pq in range(4):
            wv = cho.tile([128, 64, 10], BF16, tag="wfc")
            nc.sync.dma_start(
                out=wv, in_=din["wfct"].ap()[kg][:, pq * 640:(pq + 1) * 640])
            sl = c4[kg][:, :, 4 * pq:4 * pq + 4, :]   # pixels pq*64..+64
            if (kg * 4 + pq) % 2 == 0:
                nc.scalar.activation(sl, sl, AF.Relu, bias=ab4[kg][1][:],
                                     scale=ab4[kg][0][:])
            else:
                nc.vector.tensor_scalar(sl, sl, ab4[kg][0][:], ab4[kg][1][:],
                                        op0=AO.mult, op1=AO.add)
                nc.vector.tensor_scalar(sl, sl, 0.0, None, op0=AO.max)
            for pi in range(64):
                pix = pq * 64 + pi
                n_mm += 1
                nc.tensor.matmul(pf[:], wv[:, pi], h4v[:, :, pix],
                                 start=(n_mm == 1), stop=(n_mm == 512))
            if dbg and pq == 3:
                nc.gpsimd.dma_start(out=dbg["h4"].ap()[kg], in_=h4v)
    of = sm.tile([10, 64], F32, tag="of")
    nc.vector.tensor_copy(of[:], pf[:])
    nc.sync.dma_start(out=out_d.ap(), in_=of[:])


def _bn_apply_small(nc, sm, dram, c, sc1, sc2, nch, g_ap, b_ap, tag, dbg,
                    dbg_sg, dbg_ab):
    """c [128=(bh,c64), 32, 18, 18] f32r padded; returns same tile (h)."""
    pk = _sum_chunks(nc, sm, sc1, sc2, nch, tag)
    # combine bh pairs: pk[64:128] -> partitions 0:64 via DRAM bounce
    bounce = dram.tile([64, 2], F32, tag=tag + "bnc")
    nc.sync.dma_start(out=bounce[:], in_=pk[64:128, :])
    pk2 = sm.tile([64, 2], F32, tag=tag + "pk2")
    nc.sync.dma_start(out=pk2[:], in_=bounce[:])
    pssum = sm.tile([64, 2], F32, tag=tag + "pks")
    nc.vector.tensor_tensor(pssum[:], pk[0:64, :], pk2[:], op=AO.add)
    ar_out = _allreduce(nc, dram, pssum[:], [64, 2], tag + "ar")
    sg = sm.tile([64, 2], F32, tag=tag + "sg")
    nc.gpsimd.dma_start(out=sg[:], in_=ar_out[:])
    if dbg and dbg_sg:
        nc.sync.dma_start(out=dbg[dbg_sg].ap(), in_=sg[:])
    a64, b64 = _emit_ab(nc, sm, sg[:, 0:1], sg[:, 1:2], 131072,
                        g_ap[0:64], b_ap[0:64], tag)
    ab = sm.tile([64, 2], F32, tag=tag + "ab")
    nc.vector.tensor_copy(ab[:, 0:1], a64[:])
    nc.vector.tensor_copy(ab[:, 1:2], b64[:])
    abd = dram.tile([64, 2], F32, tag=tag + "abd")
    nc.sync.dma_start(out=abd[:], in_=ab[:])
    ab128 = sm.tile([128, 2], F32, tag=tag + "abb")
    nc.sync.dma_start(out=ab128[0:64, :], in_=abd[:])
    nc.sync.dma_start(out=ab128[64:128, :], in_=abd[:])
    if dbg and dbg_ab:
        nc.sync.dma_start(out=dbg[dbg_ab].ap(), in_=ab128[:])
    # apply in place over the interior only (borders stay zero)
    for k in range(8):
        iv = c[:, 4 * k:4 * k + 4, 1:17, 1:17]
        nc.scalar.activation(iv, iv, AF.Relu, bias=ab128[:, 1:2],
                             scale=ab128[:, 0:1])
    return c


# ------------------------------------------------------------------ kernel

def _prep_inputs(x, w1, w21, w22, w31, w32, w41, w42,
                 g1, b1, g2, b2, g3, b3, g4, b4, wfc):
    xcol2, w2t, d2, d3, d4, w22t, w32t, w42t, wfct = _host_prep(
        x, w1, w21, w31, w41, w22, w32, w42, wfc)
    f32 = lambda v: np.asarray(v, np.float32)
    g1, b1, g2, b2 = f32(g1), f32(b1), f32(g2), f32(b2)
    g3, b3, g4, b4 = f32(g3), f32(b3), f32(g4), f32(b4)
    gb = np.zeros((128, 10), np.float32)
    gb[:, 0] = np.tile(g1, 2); gb[:, 1] = np.tile(b1, 2)
    gb[:, 2] = np.tile(g2, 2); gb[:, 3] = np.tile(b2, 2)
    gb[:, 4] = g3[:128]; gb[:, 5] = g3[128:]
    gb[:, 6] = b3[:128]; gb[:, 7] = b3[128:]
    gb2 = np.zeros((128, 4), np.float32)
    gb2[:, 0] = g4[:128]; gb2[:, 1] = g4[128:]
    gb2[:, 2] = b4[:128]; gb2[:, 3] = b4[128:]
    in_maps = []
    for c in range(N_CORES):
        in_maps.append({
            "xcol2": xcol2[c], "w2t": w2t, "d2": d2, "d3": d3, "d4": d4,
            "w22t": w22t, "w32t": w32t, "w42t": w42t, "wfct": wfct,
            "gb": gb, "gb2": gb2,
        })
    return in_maps


def kernel(x, w1, w21, w22, w31, w32, w41, w42,
           g1, b1, g2, b2, g3, b3, g4, b4, wfc, bfc):
    debug = bool(int(os.environ.get("BCK_DEBUG", "0")))
    key = ("nc", debug)
    if key not in _CACHE:
        _CACHE[key] = build(debug=debug)
    nc = _CACHE[key]
    in_maps = _prep_inputs(x, w1, w21, w22, w31, w32, w41, w42,
                           g1, b1, g2, b2, g3, b3, g4, b4, wfc)
    res = bass_utils.run_bass_kernel_spmd(
        nc, in_maps, core_ids=list(range(N_CORES)))
    kernel.last_results = res
    outs = [res.results[c]["out"] for c in range(N_CORES)]
    full = np.concatenate([o.T for o in outs], axis=0)  # [512, 10]
    return (full + np.asarray(bfc, np.float32)[None, :]).astype(np.float32)
